# revision 37
# baseline (speedup 1.0000x reference)
"""Causal self-attention (B=2, T=2048, C=768, H=12) on 8 TRN2 NeuronCores.

Sharding: data-parallel over B (cores 0-3 -> b=0, cores 4-7 -> b=1), tensor
parallel over heads (3 heads per core). Each core computes q/k/v projections
for its 3 heads, causal attention, and a partial output projection; the host
sums the 4 partials per batch element and adds the output/v biases.

Attention is computed transposed: S^T[tk, tq] = K Q^T so that the softmax
denominator comes out of the ones-augmented AV matmul (V | 1) as row 64 of
the [65, 512] PSUM accumulator; exp runs on the scalar engine straight out
of PSUM with 1/sqrt(d) folded into the activation scale. Heads A and B are
packed onto PE row-strips 0-63 / 64-127 so their K=64 S-matmuls execute
concurrently. Diagonal tiles only compute their causally-valid column range.

The attention phase is ACT(exp)-bound, so the q/k/v projection matmuls for
chunk j+1 and the out-projection for chunk j-1 are injected between the
attention tile-steps of chunk j to keep PE busy while ACT drains.

The v bias never touches the device: softmax rows sum to 1, so its
contribution is the constant vector out_w @ qkv_b[2C:], added on the host.
"""

import numpy as np
import ml_dtypes
from contextlib import ExitStack

import concourse.bass as bass
import concourse.tile as tile
from concourse import bacc, mybir
from concourse.bass_utils import run_bass_kernel_spmd

BF16 = mybir.dt.bfloat16
F32 = mybir.dt.float32
AF = mybir.ActivationFunctionType

B, T, C, H, D = 2, 2048, 768, 12, 64
HPC = 3          # heads per core
NCORES = 8
CC = C // 128    # 6 contraction chunks
NT = T // 128    # 16 t tiles
NJ = T // 512    # 4 tq chunks
VW = D + 1       # 65: v columns + ones column
SCALE = float(D) ** -0.5

_cache = {}


def _build_program():
    nc = bacc.Bacc("TRN2", target_bir_lowering=False, debug=False,
                   enable_asserts=False, num_devices=NCORES)

    xt_d = nc.dram_tensor("xt_s", [128, CC * T], BF16, kind="ExternalInput").ap()
    wqk_d = nc.dram_tensor("wqk_s", [128, CC * 384], BF16, kind="ExternalInput").ap()
    wv_d = nc.dram_tensor("wv_s", [128, CC * 192], BF16, kind="ExternalInput").ap()
    bqk_d = nc.dram_tensor("bqk_s", [128, 3], F32, kind="ExternalInput").ap()
    w2ab_d = nc.dram_tensor("w2ab_s", [128, C], BF16, kind="ExternalInput").ap()
    w2c_d = nc.dram_tensor("w2c_s", [64, C], BF16, kind="ExternalInput").ap()
    masks_d = nc.dram_tensor("masks_s", [128, 4 * 512], BF16, kind="ExternalInput").ap()
    outp_d = nc.dram_tensor("outp", [128, CC * T], F32, kind="ExternalOutput").ap()

    with tile.TileContext(nc) as tc, ExitStack() as ctx:
        const = ctx.enter_context(tc.tile_pool(name="const", bufs=1))
        big = ctx.enter_context(tc.tile_pool(name="big", bufs=1))
        psum = ctx.enter_context(tc.tile_pool(name="psum", bufs=4, space="PSUM"))
        psum_pr = ctx.enter_context(tc.tile_pool(name="psum_pr", bufs=2, space="PSUM"))
        psum_av = ctx.enter_context(tc.tile_pool(name="psum_av", bufs=2, space="PSUM"))
        ppool = ctx.enter_context(tc.tile_pool(name="ppool", bufs=6))
        small = ctx.enter_context(tc.tile_pool(name="small", bufs=4))

        # warm the ACT exp table while DMAs are in flight
        warm = small.tile([1, 16], F32, tag="warm")
        nc.vector.memset(warm[:], 0.0)
        warm2 = small.tile([1, 16], F32, tag="warm")
        nc.scalar.activation(warm2[:], warm[:], AF.Exp)

        # ---- load constants/inputs (order matters: earliest consumers first)
        wqk = const.tile([128, CC * 384], BF16)
        nc.sync.dma_start(wqk[:], wqk_d[:])
        bqk = const.tile([128, 3], F32)
        nc.sync.dma_start(bqk[:], bqk_d[:])
        # xt loaded tq-chunk-major so the first qk group only waits ~4us
        xt = const.tile([128, CC * T], BF16)
        for tj in range(2):
            for kc in range(CC):
                sl = slice(kc * T + tj * 1024, kc * T + (tj + 1) * 1024)
                nc.sync.dma_start(xt[:, sl], xt_d[:, sl])
        wv = const.tile([128, CC * 192], BF16)
        nc.sync.dma_start(wv[:], wv_d[:])
        masks = const.tile([128, 4 * 512], BF16)
        nc.sync.dma_start(masks[:], masks_d[:])
        w2ab = const.tile([128, C], BF16)
        nc.sync.dma_start(w2ab[:], w2ab_d[:])
        w2c = const.tile([64, C], BF16)
        nc.sync.dma_start(w2c[:], w2c_d[:])

        # ---- persistent intermediates
        qt1 = big.tile([128, T], BF16)   # qA (p 0-63) | qB (p 64-127), [d, t]
        kt1 = big.tile([128, T], BF16)   # kA | kB
        qkt2 = big.tile([128, T], BF16)  # qC | kC
        kt2 = big.tile([64, T], BF16)    # kC shifted to partitions 0-63
        vbuf = big.tile([128, NT * HPC * VW], BF16)  # per t-chunk: [vA 1|vB 1|vC 1]
        ot_ab = big.tile([128, T], BF16)  # O.T heads A,B (out-proj rhs chunk 0)
        ot_c = big.tile([64, T], BF16)    # O.T head C   (out-proj rhs chunk 1)

        nc.vector.memset(vbuf[:], 1.0)

        def s_operands(h, i, j, c0):
            """(lhsT=k-tile, rhs=q-chunk) for head h, tk-tile i, tq-chunk j.
            Head C alternates PE row strips so adjacent tiles run
            concurrently (kC already sits at partitions 64-127 of qkt2)."""
            it = slice(i * 128, (i + 1) * 128)
            qs = slice(j * 512 + c0, (j + 1) * 512)
            if h == 0:
                return kt1[0:64, it], qt1[0:64, qs]
            if h == 1:
                return kt1[64:128, it], qt1[64:128, qs]
            return kt2[0:64, it], qkt2[0:64, qs]

        def v_ap(h, i):
            off = i * HPC * VW + h * VW
            return vbuf[:, off:off + VW]

        qk_dest = [qt1, kt1, qkt2]

        def emit_qk_group(jt, j):
            ps = psum_pr.tile([128, 512], F32, tag="proj", name=f"qk_{jt}_{j}")
            for kc in range(CC):
                nc.tensor.matmul(
                    ps[:],
                    wqk[:, kc * 384 + jt * 128: kc * 384 + (jt + 1) * 128],
                    xt[:, kc * T + j * 512: kc * T + (j + 1) * 512],
                    start=(kc == 0), stop=(kc == CC - 1),
                )
            nc.vector.tensor_scalar_add(
                qk_dest[jt][:, j * 512:(j + 1) * 512], ps[:], bqk[:, jt:jt + 1])

        def emit_shift(j):
            nc.sync.dma_start(kt2[:, j * 512:(j + 1) * 512],
                              qkt2[64:128, j * 512:(j + 1) * 512])

        def emit_v_group(ti):
            ps = psum_pr.tile([128, 192], F32, tag="proj", name=f"v_{ti}")
            for kc in range(CC):
                nc.tensor.matmul(
                    ps[:],
                    xt[:, kc * T + ti * 128: kc * T + (ti + 1) * 128],
                    wv[:, kc * 192:(kc + 1) * 192],
                    start=(kc == 0), stop=(kc == CC - 1),
                )
            dst = vbuf[:, ti * HPC * VW:(ti + 1) * HPC * VW]
            dst = dst.rearrange("p (h x) -> p h x", h=HPC)[:, :, 0:D]
            nc.vector.tensor_copy(
                dst, ps[:].rearrange("p (h x) -> p h x", h=HPC))

        def emit_outproj_group(jt, j):
            js = slice(j * 512, (j + 1) * 512)
            ps = psum_pr.tile([128, 512], F32, tag="proj", name=f"op_{jt}_{j}")
            nc.tensor.matmul(ps[:], w2ab[:, jt * 128:(jt + 1) * 128],
                             ot_ab[:, js], start=True, stop=False)
            nc.tensor.matmul(ps[:], w2c[:, jt * 128:(jt + 1) * 128],
                             ot_c[:, js], start=False, stop=True)
            ob = small.tile([128, 512], F32, tag="ob", name=f"ob_{jt}_{j}")
            nc.vector.tensor_copy(ob[:], ps[:])
            nc.sync.dma_start(
                outp_d[:, jt * T + j * 512: jt * T + (j + 1) * 512], ob[:])

        # warm-up matmuls: PE activity during the input DMA wait so the HAM
        # clock-gate reaches K=8/8 before real work starts
        wz = const.tile([128, 512], BF16, name="wz")
        nc.vector.memset(wz[:], 0.0)
        wps = psum.tile([128, 512], F32, tag="s", name="wps")
        for _ in range(10):
            nc.tensor.matmul(wps[:], wz[:, 0:128], wz[:], start=True, stop=True)

        # prologue: just enough projections to start attention chunk 0
        emit_qk_group(0, 0)
        emit_qk_group(1, 0)
        emit_v_group(0)
        emit_v_group(1)

        for j in range(NJ):
            js = slice(j * 512, (j + 1) * 512)
            n_i = 4 * j + 4

            # work to inject between attention tile-steps of this chunk
            inj = []
            if j == 0:
                inj += [lambda: emit_v_group(2), lambda: emit_v_group(3),
                        lambda: emit_qk_group(2, 0), lambda: emit_shift(0)]
            if j + 1 < NJ:
                inj += [lambda jt=jt: emit_qk_group(jt, j + 1) for jt in range(3)]
                inj.append(lambda: emit_shift(j + 1))
                inj += [lambda ti=ti: emit_v_group(ti)
                        for ti in range(4 * (j + 1), 4 * (j + 1) + 4)]
            if j - 1 >= 0:
                inj += [lambda jt=jt: emit_outproj_group(jt, j - 1)
                        for jt in range(CC)]

            def emit_dummy():
                # keeps the HAM clock-gate at K=8/8 through sparse stretches
                dps = psum.tile([128, 512], F32, tag="s", name="dps")
                for _ in range(2):
                    nc.tensor.matmul(dps[:], wz[:, 0:128], wz[:],
                                     start=True, stop=True)

            if j == NJ - 1:
                inj += [emit_dummy for _ in range(8)]
            total_steps = 4 * j + 4   # inject slots over both head groups
            step = 0

            def maybe_inject():
                nonlocal step
                step += 1
                remaining_slots = total_steps - step + 1
                k = -(-len(inj) // max(1, remaining_slots))  # ceil
                for _ in range(min(k, len(inj))):
                    inj.pop(0)()

            for group in ((0, 1), (2,)):
                av = {h: psum_av.tile([VW, 512], F32, tag="av",
                                      name=f"av_{h}_{j}") for h in group}
                started = {h: False for h in group}
                prev = []   # (h, i, pt_ap, col0) from previous tile-step

                def flush_av(last=False):
                    for (h, i, pt_ap, c0) in prev:
                        nc.tensor.matmul(
                            av[h][:, c0:512], v_ap(h, i), pt_ap,
                            start=(not started[h]), stop=last,
                            skip_group_check=True,
                        )
                        started[h] = True
                    prev.clear()

                # full tiles (i < 4j)
                for i0 in range(4 * j):
                    sp2 = {h: psum.tile([128, 512], F32, tag="s",
                                        name=f"sp2_{h}_{j}_{i0}")
                           for h in group}
                    for h in group:
                        lhsT, rhs = s_operands(h, i0, j, 0)
                        nc.tensor.matmul(sp2[h][:], lhsT, rhs,
                                         start=True, stop=True)
                    new = []
                    for h in group:
                        pt2 = ppool.tile([128, 512], BF16, tag="pt",
                                         name=f"pt2_{h}_{j}_{i0}")
                        nc.scalar.activation(pt2[:], sp2[h][:], AF.Exp,
                                             scale=SCALE)
                        new.append((h, i0, pt2[:], 0))
                    flush_av()
                    prev.extend(new)
                    if i0 % 2 == 1:
                        maybe_inject()

                # diagonal tiles i = 4j+oi, restricted to cols >= 128*oi
                for oi in range(4):
                    i = 4 * j + oi
                    c0 = 128 * oi
                    w = 512 - c0
                    sp = {h: psum.tile([128, w], F32, tag="s",
                                       name=f"sp_{h}_{j}_{oi}")
                          for h in group}
                    for h in group:
                        lhsT, rhs = s_operands(h, i, j, c0)
                        nc.tensor.matmul(sp[h][:], lhsT, rhs,
                                         start=True, stop=True)
                    new = []
                    for h in group:
                        pt = ppool.tile([128, w], BF16, tag="pt",
                                        name=f"pt_{h}_{j}_{oi}")
                        nc.scalar.activation(pt[:], sp[h][:], AF.Exp,
                                             scale=SCALE)
                        nc.vector.tensor_mul(
                            pt[:], pt[:],
                            masks[:, oi * 512 + c0:(oi + 1) * 512])
                        new.append((h, i, pt[:], c0))
                    flush_av()
                    prev.extend(new)
                    if oi % 2 == 1:
                        maybe_inject()
                flush_av(last=True)

                # normalize: O.T[d, tq] = av[0:64] / av[64]
                for h in group:
                    recip = small.tile([1, 512], F32, tag="recip",
                                       name=f"recip_{h}_{j}")
                    den = small.tile([1, 512], F32, tag="den",
                                     name=f"den_{h}_{j}")
                    nc.vector.tensor_copy(den[:], av[h][D:VW, :])
                    # custom-DVE ops read garbage from PSUM; SBUF source only
                    nc.vector.reciprocal_approx_fast(recip[:], den[:])
                    rb = small.tile([64, 512], F32, tag="rb", name=f"rb_{h}_{j}")
                    nc.gpsimd.partition_broadcast(rb[:], recip[:])
                    if h == 0:
                        dst = ot_ab[0:64, js]
                    elif h == 2:
                        dst = ot_c[0:64, js]
                    else:
                        dst = small.tile([64, 512], BF16, tag="otb",
                                         name=f"otb_{j}")
                    nc.vector.tensor_mul(dst[:], av[h][0:D, :], rb[:])
                    if h == 1:
                        nc.sync.dma_start(ot_ab[64:128, js], dst[:])

            # leftover injections for this chunk
            while inj:
                inj.pop(0)()

        # epilogue: out-projection for the last chunk, with PE kept warm
        for jt in range(CC):
            emit_outproj_group(jt, NJ - 1)
            dps = psum.tile([128, 512], F32, tag="s", name=f"dps_e{jt}")
            for _ in range(3):
                nc.tensor.matmul(dps[:], wz[:, 0:128], wz[:],
                                 start=True, stop=True)

    nc.compile()
    return nc


def _prep_in_maps(x, qkv_w, qkv_b, out_w):
    bf = ml_dtypes.bfloat16
    in_maps = []

    # causal masks for the 4 diagonal offsets: keep when f >= oi*128 + p
    p = np.arange(128)[:, None]
    f = np.arange(512)[None, :]
    masks = np.stack([(f >= oi * 128 + p) for oi in range(4)])  # [4,128,512]
    masks_s = np.ascontiguousarray(
        masks.transpose(1, 0, 2).reshape(128, 4 * 512)).astype(bf)

    for c in range(NCORES):
        b = c // 4
        h0 = (c % 4) * HPC
        hs = [h0, h0 + 1, h0 + 2]

        xT = np.ascontiguousarray(x[b].T.astype(np.float32))  # [768, 2048]
        xt_s = xT.reshape(CC, 128, T).transpose(1, 0, 2).reshape(128, CC * T)

        qr = lambda h: qkv_w[h * D:(h + 1) * D]
        kr = lambda h: qkv_w[C + h * D: C + (h + 1) * D]
        vr = lambda h: qkv_w[2 * C + h * D: 2 * C + (h + 1) * D]
        qb = lambda h: qkv_b[h * D:(h + 1) * D]
        kb = lambda h: qkv_b[C + h * D: C + (h + 1) * D]

        wqk = np.concatenate([qr(hs[0]), qr(hs[1]), kr(hs[0]), kr(hs[1]),
                              qr(hs[2]), kr(hs[2])], axis=0)  # [384, 768]
        wqk_s = np.ascontiguousarray(wqk.T).reshape(CC, 128, 384) \
            .transpose(1, 0, 2).reshape(128, CC * 384)
        wv_ = np.concatenate([vr(h) for h in hs], axis=0)      # [192, 768]
        wv_s = np.ascontiguousarray(wv_.T).reshape(CC, 128, 192) \
            .transpose(1, 0, 2).reshape(128, CC * 192)

        bqk = np.concatenate([qb(hs[0]), qb(hs[1]), kb(hs[0]), kb(hs[1]),
                              qb(hs[2]), kb(hs[2])])
        bqk_s = np.ascontiguousarray(bqk.reshape(3, 128).T).astype(np.float32)

        ch_ab = np.r_[hs[0] * D:(hs[0] + 1) * D, hs[1] * D:(hs[1] + 1) * D]
        ch_c = np.r_[hs[2] * D:(hs[2] + 1) * D]
        w2ab_s = np.ascontiguousarray(out_w[:, ch_ab].T)  # [128, 768]
        w2c_s = np.ascontiguousarray(out_w[:, ch_c].T)    # [64, 768]

        in_maps.append({
            "xt_s": np.ascontiguousarray(xt_s).astype(bf),
            "wqk_s": np.ascontiguousarray(wqk_s).astype(bf),
            "wv_s": np.ascontiguousarray(wv_s).astype(bf),
            "bqk_s": bqk_s,
            "w2ab_s": w2ab_s.astype(bf),
            "w2c_s": w2c_s.astype(bf),
            "masks_s": masks_s,
        })
    return in_maps


def _assemble(results, qkv_b, out_w, out_b):
    out = np.zeros((B, T, C), dtype=np.float32)
    for c in range(NCORES):
        b = c // 4
        outp = results[c]["outp"]  # [128, CC*T] f32
        outT = outp.reshape(128, CC, T).transpose(1, 0, 2).reshape(C, T)
        out[b] += outT.T
    # v-bias contribution (softmax rows sum to 1) + output bias
    const = out_w.astype(np.float32) @ qkv_b[2 * C:].astype(np.float32) \
        + out_b.astype(np.float32)
    out += const[None, None, :]
    return out


def run(x, qkv_w, qkv_b, out_w, out_b, trace=False, tmpdir=None):
    if "nc" not in _cache:
        _cache["nc"] = _build_program()
    nc = _cache["nc"]
    x = np.asarray(x, dtype=np.float32)
    qkv_w = np.asarray(qkv_w, dtype=np.float32)
    qkv_b = np.asarray(qkv_b, dtype=np.float32)
    out_w = np.asarray(out_w, dtype=np.float32)
    out_b = np.asarray(out_b, dtype=np.float32)
    in_maps = _prep_in_maps(x, qkv_w, qkv_b, out_w)
    res = run_bass_kernel_spmd(nc, in_maps, list(range(NCORES)), trace=trace,
                               tmpdir=tmpdir)
    out = _assemble(res.results, qkv_b, out_w, out_b)
    return out, res


def kernel(x, qkv_w, qkv_b, out_w, out_b):
    out, _ = run(x, qkv_w, qkv_b, out_w, out_b, trace=False)
    return out
